# revision 4
# baseline (speedup 1.0000x reference)
"""Multi-head causal attention on 8 Trainium2 NeuronCores (Bass/Tile).

Problem: B=2, S=2048, D=1024, H=16 heads (HD=64). Reference returns
(out [B,S,D] f32, probs [B,H,S,S] f32).

Sharding (data + head parallel): core c in 0..7 handles batch b=c//4 and
head-group hg=c%4 (4 of 16 heads). Each core:
  - projects Q^T/K^T per head ([64, 2048] bf16, d-on-partitions) and
    V ([2048, 256] bf16) from host-pre-transposed bf16 activations and
    column-sliced weights,
  - computes scores = (Q/8).K^T per head with block sparsity derived from
    the mask (512-wide column blocks classified zero/ones/mixed),
  - softmax WITHOUT max-subtraction (scores are O(1) here, exp cannot
    overflow); mask applied as a post-exp multiply so masked probs are
    exactly 0, matching the reference's exp(-65504)->0,
  - writes its [4, S, S] f32 probs slice (fully-masked blocks are skipped;
    output buffers are pre-zeroed by the runtime),
  - ctx via DMA-transposed bf16 probs against V, then row-sliced output
    projection -> partial out^T [D, S] f32, summed + transposed on host.

No collectives; host gathers/assembles the full outputs.
"""

import os
import numpy as np
import ml_dtypes

import concourse.bass as bass
import concourse.mybir as mybir
import concourse.tile as tile
from concourse.bass_utils import run_bass_kernel_spmd

B, S, D, H = 2, 2048, 1024, 16
HD = D // H            # 64
NCORES = 8
HPC = 4                # heads per core
CD = HPC * HD          # 256 ctx dims per core
NT = S // 128          # 16 q-tiles of 128 rows
NC512 = S // 512       # 4 column blocks of 512
NJ = S // 128          # 16 column k-tiles of 128
FP16_MIN = -65504.0

BF16 = mybir.dt.bfloat16
F32 = mybir.dt.float32
NPBF16 = ml_dtypes.bfloat16
AF = mybir.ActivationFunctionType
ALU = mybir.AluOpType
AX = mybir.AxisListType


def _split_excess_waits(nc):
    """walrus in this container rejects >1 sync-wait per instruction
    ("Too many sync wait commands" in CoreV3 setupSyncWait). Move excess
    waits onto NoOps inserted just before the offending instruction."""
    n = 0
    for fn in nc.m.functions:
        for blk in fn.blocks:
            out = []
            for inst in blk.instructions:
                si = inst.sync_info
                if si is not None and si.on_wait and len(si.on_wait) > 1:
                    waits = list(si.on_wait)
                    for w in waits[:-1]:
                        nop = mybir.InstNoOp(name=f"WSPLIT{n}", ins=[], outs=[])
                        n += 1
                        nop.engine = inst.engine
                        nop.sync_info = mybir.SyncInfo(on_wait=[w], on_update=[])
                        out.append(nop)
                    si.on_wait = [waits[-1]]
                out.append(inst)
            blk.instructions[:] = out
    return nc


def _classify(mask01):
    """mask01: [B, S, S] bool. Returns (cls [NT,NC512] in {0,1,2},
    a128 [NT,NJ] bool), merged across batches so one SPMD program
    serves every core."""
    tb = mask01.reshape(B, NT, 128, NC512, 512)
    anyb = tb.any(axis=(2, 4)).any(axis=0)        # [NT, NC512]
    allb = tb.all(axis=(2, 4)).all(axis=0)
    cls = np.where(anyb, np.where(allb, 1, 2), 0).astype(np.int64)
    a128 = mask01.reshape(B, NT, 128, NJ, 128).any(axis=(0, 2, 4))  # [NT, NJ]
    return cls, a128


def _runs(cols):
    """Maximal runs of consecutive ints: [0,1,3] -> [(0,2),(3,4)]."""
    runs = []
    for c in cols:
        if runs and runs[-1][1] == c:
            runs[-1][1] = c + 1
        else:
            runs.append([c, c + 1])
    return [tuple(r) for r in runs]


def build_program(cls, a128, nmix):
    cls = np.asarray(cls)
    a128 = np.asarray(a128)
    active512 = [[c for c in range(NC512) if cls[t, c]] for t in range(NT)]
    mixed_idx = {}
    for t in range(NT):
        for c in range(NC512):
            if cls[t, c] == 2:
                mixed_idx[(t, c)] = len(mixed_idx)
    assert len(mixed_idx) == nmix
    jgroups = [
        sorted({j for t in range(4 * g, 4 * g + 4) for j in range(NJ) if a128[t, j]})
        for g in range(4)
    ]

    nc = bass.Bass()
    xqT = nc.dram_tensor("xqT", [D, S], BF16, kind="ExternalInput")
    xkT = nc.dram_tensor("xkT", [D, S], BF16, kind="ExternalInput")
    xvT = nc.dram_tensor("xvT", [D, S], BF16, kind="ExternalInput")
    wq = nc.dram_tensor("wq", [D, CD], BF16, kind="ExternalInput")
    wk = nc.dram_tensor("wk", [D, CD], BF16, kind="ExternalInput")
    wv = nc.dram_tensor("wv", [D, CD], BF16, kind="ExternalInput")
    wo = nc.dram_tensor("wo", [CD, D], BF16, kind="ExternalInput")
    bqs = nc.dram_tensor("bqs", [CD], F32, kind="ExternalInput")   # bq slice * 0.125
    bks = nc.dram_tensor("bks", [CD], F32, kind="ExternalInput")
    bvs = nc.dram_tensor("bvs", [CD], F32, kind="ExternalInput")
    maskb = None
    if nmix:
        maskb = nc.dram_tensor("maskb", [nmix, 128, 512], BF16, kind="ExternalInput")
    probs_o = nc.dram_tensor("probs", [HPC, S, S], F32, kind="ExternalOutput")
    outT_o = nc.dram_tensor("outT", [D, S], F32, kind="ExternalOutput")

    with tile.TileContext(nc) as tc:
        with (
            tc.tile_pool(name="persist", bufs=1) as pp,
            tc.tile_pool(name="qkv", bufs=1) as qkvp,
        ):
            # ---- persistent loads -------------------------------------
            wq_sb = pp.tile([128, 8, CD], BF16)
            nc.sync.dma_start(out=wq_sb, in_=wq[:, :].rearrange("(a p) n -> p a n", p=128))
            wk_sb = pp.tile([128, 8, CD], BF16)
            nc.sync.dma_start(out=wk_sb, in_=wk[:, :].rearrange("(a p) n -> p a n", p=128))
            wv_sb = pp.tile([128, 8, CD], BF16)
            nc.sync.dma_start(out=wv_sb, in_=wv[:, :].rearrange("(a p) n -> p a n", p=128))
            wo_sb = pp.tile([64, HPC, D], BF16)
            nc.sync.dma_start(out=wo_sb, in_=wo[:, :].rearrange("(a p) n -> p a n", p=64))
            bq_sb = pp.tile([64, HPC], F32)
            nc.sync.dma_start(out=bq_sb, in_=bqs[:].rearrange("(a p) -> p a", p=64))
            bk_sb = pp.tile([64, HPC], F32)
            nc.sync.dma_start(out=bk_sb, in_=bks[:].rearrange("(a p) -> p a", p=64))
            bv_sb = pp.tile([128, CD], F32)
            nc.sync.dma_start(out=bv_sb, in_=bvs[None, :].to_broadcast((128, CD)))
            mask_sb = None
            if nmix:
                mask_sb = pp.tile([128, nmix, 512], BF16)
                nc.sync.dma_start(out=mask_sb, in_=maskb[:, :, :].rearrange("a p n -> p a n"))

            # per-head Q^T/K^T [64, S] bf16, V [128, 16, CD] bf16, ctx^T [64, S] bf16
            qt = [qkvp.tile([64, S], BF16, name=f"qt{h}") for h in range(HPC)]
            ktl = [qkvp.tile([64, S], BF16, name=f"kt{h}") for h in range(HPC)]
            v_sb = qkvp.tile([128, NJ, CD], BF16)
            ctxT = [qkvp.tile([64, S], BF16, name=f"ctxT{h}") for h in range(HPC)]

            # ---- phase A: projections ---------------------------------
            with (
                tc.tile_pool(name="xin", bufs=2) as xp,
                tc.psum_pool(name="psA", bufs=4) as psA,
                tc.psum_pool(name="psV", bufs=2) as psV,
            ):
                for xin, wsb, bsb, outs, scale in (
                    (xqT, wq_sb, bq_sb, qt, 0.125),
                    (xkT, wk_sb, bk_sb, ktl, 1.0),
                ):
                    x_sb = xp.tile([128, 8, S], BF16, name="x_sb")
                    nc.sync.dma_start(out=x_sb, in_=xin[:, :].rearrange("(a p) n -> p a n", p=128))
                    for ng in range(4):
                        for h in range(HPC):
                            ps = psA.tile([64, 512], F32, name="psqk")
                            for kt in range(8):
                                nc.tensor.matmul(
                                    ps,
                                    wsb[:, kt, h * 64:(h + 1) * 64],
                                    x_sb[:, kt, ng * 512:(ng + 1) * 512],
                                    start=(kt == 0),
                                    stop=(kt == 7),
                                )
                            nc.scalar.activation(
                                out=outs[h][:, ng * 512:(ng + 1) * 512],
                                in_=ps,
                                func=AF.Identity,
                                bias=bsb[:, h:h + 1],
                                scale=scale,
                            )
                # V projection: natural layout [keys, CD]
                xv_sb = xp.tile([128, 8, S], BF16, name="x_sb")
                nc.sync.dma_start(out=xv_sb, in_=xvT[:, :].rearrange("(a p) n -> p a n", p=128))
                for j in range(NJ):
                    ps = psV.tile([128, CD], F32, name="psv")
                    for kt in range(8):
                        nc.tensor.matmul(
                            ps,
                            xv_sb[:, kt, j * 128:(j + 1) * 128],
                            wv_sb[:, kt, :],
                            start=(kt == 0),
                            stop=(kt == 7),
                        )
                    nc.vector.tensor_add(v_sb[:, j, :], ps, bv_sb)

            # ---- phase B: attention -----------------------------------
            # per-t contiguous runs of active 128-wide k-tiles (for batched
            # DMA transposes); inactive (t, j) slots get zero-filled.
            a128runs = [
                _runs([j for j in range(NJ) if a128[t, j]]) for t in range(NT)
            ]
            with (
                tc.tile_pool(name="work", bufs=3) as wkp,
                tc.tile_pool(name="small", bufs=8) as smp,
                tc.tile_pool(name="pt", bufs=2) as ptp,
                tc.psum_pool(name="psB", bufs=4) as psB,
                tc.psum_pool(name="psC", bufs=2) as psC,
            ):
                for h in range(HPC):
                    for g in range(4):
                        jg = jgroups[g]
                        # probs^T for this q-group: [k-part, j, tl, q128]
                        # slot (j, tl) holds Pb[tl-th q-tile][:, j*128:...].T
                        PT = ptp.tile([128, NJ, 4, 128], BF16, name="PT", tag="PT")
                        for t in range(4 * g, 4 * g + 4):
                            tl = t - 4 * g
                            acts = active512[t]
                            Pb = None
                            if acts:
                                E = wkp.tile([128, S], BF16, name="E", tag="E")
                                sums = smp.tile([128, NC512], F32, name="sums", tag="sums")
                                for ci, c in enumerate(acts):
                                    sc = psB.tile([128, 512], F32, name="sc", tag="sc")
                                    nc.tensor.matmul(
                                        sc,
                                        qt[h][:, t * 128:(t + 1) * 128],
                                        ktl[h][:, c * 512:(c + 1) * 512],
                                        start=True,
                                        stop=True,
                                    )
                                    nc.scalar.activation(
                                        out=E[:, c * 512:(c + 1) * 512],
                                        in_=sc,
                                        func=AF.Exp,
                                        accum_out=sums[:, ci:ci + 1],
                                    )
                                for ci, c in enumerate(acts):
                                    if cls[t, c] == 2:
                                        mi = mixed_idx[(t, c)]
                                        cs = slice(c * 512, (c + 1) * 512)
                                        nc.vector.scalar_tensor_tensor(
                                            out=E[:, cs],
                                            in0=E[:, cs],
                                            scalar=0.0,
                                            in1=mask_sb[:, mi, :],
                                            op0=ALU.bypass,
                                            op1=ALU.mult,
                                            accum_out=sums[:, ci:ci + 1],
                                        )
                                stot = smp.tile([128, 1], F32, name="stot", tag="stot")
                                nc.vector.reduce_sum(stot, sums[:, 0:len(acts)], axis=AX.X)
                                rec = smp.tile([128, 1], F32, name="rec", tag="rec")
                                nc.vector.reciprocal(rec, stot)
                                Pb = wkp.tile([128, S], BF16, name="Pb", tag="Pb")
                                Pf = wkp.tile([128, S], F32, name="Pf", tag="Pf")
                                for c0, c1 in _runs(acts):
                                    rs = slice(c0 * 512, c1 * 512)
                                    nc.vector.tensor_scalar_mul(Pb[:, rs], E[:, rs], rec)
                                    nc.vector.tensor_scalar_mul(Pf[:, rs], E[:, rs], rec)
                                    nc.sync.dma_start(
                                        out=probs_o[h, t * 128:(t + 1) * 128, rs],
                                        in_=Pf[:, rs],
                                    )
                            # batched per-block transpose: out[:, j, tl, :] =
                            # Pb[:, j*128:(j+1)*128].T for each active j
                            for j0, j1 in (a128runs[t] if acts else []):
                                nc.sync.dma_start_transpose(
                                    out=PT[:, j0:j1, tl, :],
                                    in_=Pb[:, j0 * 128:j1 * 128],
                                )
                            # zero-fill inactive (j, tl) slots among jg
                            inact = [j for j in jg if not (acts and a128[t, j])]
                            for j0, j1 in _runs(inact):
                                nc.vector.memset(PT[:, j0:j1, tl, :], 0.0)
                        # ctx^T[h, g] = sum_j V[j,h].T @ probsT[j]
                        gs = slice(g * 512, (g + 1) * 512)
                        if jg:
                            cps = psC.tile([64, 512], F32, name="cps", tag="cps")
                            for idx, j in enumerate(jg):
                                nc.tensor.matmul(
                                    cps,
                                    v_sb[:, j, h * 64:(h + 1) * 64],
                                    PT[:, j, :, :],
                                    start=(idx == 0),
                                    stop=(idx == len(jg) - 1),
                                )
                            nc.scalar.activation(out=ctxT[h][:, gs], in_=cps, func=AF.Copy)
                        else:
                            nc.vector.memset(ctxT[h][:, gs], 0.0)

            # ---- phase C: output projection ---------------------------
            with (
                tc.tile_pool(name="oute", bufs=4) as op,
                tc.psum_pool(name="psD", bufs=4) as psD,
            ):
                for m in range(8):
                    for g2 in range(4):
                        po = psD.tile([128, 512], F32, name="po", tag="po")
                        for h in range(HPC):
                            nc.tensor.matmul(
                                po,
                                wo_sb[:, h, m * 128:(m + 1) * 128],
                                ctxT[h][:, g2 * 512:(g2 + 1) * 512],
                                start=(h == 0),
                                stop=(h == HPC - 1),
                            )
                        ot = op.tile([128, 512], F32, name="ot", tag="ot")
                        nc.scalar.activation(out=ot, in_=po, func=AF.Copy)
                        nc.sync.dma_start(
                            out=outT_o[m * 128:(m + 1) * 128, g2 * 512:(g2 + 1) * 512],
                            in_=ot,
                        )

    return _split_excess_waits(nc)


_prog_cache = {}


def kernel(x_q, x_k, x_v, mask, Wq, bq, Wk, bk, Wv, bv, Wo, bo):
    x_q = np.asarray(x_q, np.float32)
    x_k = np.asarray(x_k, np.float32)
    x_v = np.asarray(x_v, np.float32)
    mask = np.asarray(mask)
    Wq = np.asarray(Wq, np.float32)
    Wk = np.asarray(Wk, np.float32)
    Wv = np.asarray(Wv, np.float32)
    Wo = np.asarray(Wo, np.float32)
    bq = np.asarray(bq, np.float32)
    bk = np.asarray(bk, np.float32)
    bv = np.asarray(bv, np.float32)
    bo = np.asarray(bo, np.float32)

    mask01 = (mask != 0).reshape(B, S, S)
    cls, a128 = _classify(mask01)
    mixed = [(t, c) for t in range(NT) for c in range(NC512) if cls[t, c] == 2]
    nmix = len(mixed)

    key = (cls.tobytes(), a128.tobytes())
    if key not in _prog_cache:
        _prog_cache[key] = build_program(cls, a128, nmix)
    nc = _prog_cache[key]

    # host-side sharding / preprocessing
    xT = {}
    for name, x in (("xqT", x_q), ("xkT", x_k), ("xvT", x_v)):
        xT[name] = [np.ascontiguousarray(x[b].T).astype(NPBF16) for b in range(B)]
    if nmix:
        maskb = [
            np.stack(
                [
                    mask01[b, t * 128:(t + 1) * 128, c * 512:(c + 1) * 512]
                    for (t, c) in mixed
                ]
            ).astype(NPBF16)
            for b in range(B)
        ]

    in_maps = []
    for c in range(NCORES):
        b, hg = c // 4, c % 4
        cs = slice(hg * CD, (hg + 1) * CD)
        d = {
            "xqT": xT["xqT"][b],
            "xkT": xT["xkT"][b],
            "xvT": xT["xvT"][b],
            "wq": np.ascontiguousarray(Wq[:, cs]).astype(NPBF16),
            "wk": np.ascontiguousarray(Wk[:, cs]).astype(NPBF16),
            "wv": np.ascontiguousarray(Wv[:, cs]).astype(NPBF16),
            "wo": np.ascontiguousarray(Wo[cs, :]).astype(NPBF16),
            "bqs": np.ascontiguousarray(bq[cs]) * np.float32(0.125),
            "bks": np.ascontiguousarray(bk[cs]),
            "bvs": np.ascontiguousarray(bv[cs]),
        }
        if nmix:
            d["maskb"] = maskb[b]
        in_maps.append(d)

    trace = bool(int(os.environ.get("KERNEL_TRACE", "0")))
    res = run_bass_kernel_spmd(nc, in_maps, core_ids=list(range(NCORES)), trace=trace)
    if trace and res.exec_time_ns is not None:
        print(f"HW exec time: {res.exec_time_ns} ns")
        kernel.last_exec_time_ns = res.exec_time_ns
        kernel.last_trace = res.instructions_and_trace

    probs = np.zeros((B, H, S, S), np.float32)
    out = np.zeros((B, S, D), np.float32)
    for c in range(NCORES):
        b, hg = c // 4, c % 4
        r = res.results[c]
        probs[b, hg * HPC:(hg + 1) * HPC] = r["probs"]
        out[b] += r["outT"].T
    out += bo
    return out, probs


# revision 12
# speedup vs baseline: 1.8856x; 1.8856x over previous
"""Multi-head causal attention on 8 Trainium2 NeuronCores (Bass/Tile).

Problem: B=2, S=2048, D=1024, H=16 heads (HD=64). Reference returns
(out [B,S,D] f32, probs [B,H,S,S] f32).

Sharding (data + head parallel): core c in 0..7 handles batch b=c//4 and
head-group hg=c%4 (4 of 16 heads). Each core:
  - projects Q^T/K^T head-PAIRED ([128, 2048] bf16: pair p holds heads
    2p,2p+1 on partition halves; d-on-partitions) and V ([2048, 256] bf16)
    from host-pre-transposed bf16 activations and column-sliced weights,
  - computes scores = (Q/8).K^T per head (K=64 matmuls reading the pair
    tile's partition half) with block sparsity from the mask (512-wide
    column blocks classified zero/ones/mixed),
  - softmax WITHOUT max-subtraction (scores are O(1) here, exp cannot
    overflow); mask applied as a post-exp multiply so masked probs are
    exactly 0, matching the reference's exp(-65504)->0,
  - writes its [4, S, S] f32 probs slice (fully-masked blocks are skipped;
    output buffers are pre-zeroed by the runtime),
  - ctx via batched DMA-transposed bf16 probs against V (both pair-heads
    accumulate into one PSUM tile), then K=128 output projection with
    row-sliced Wo fused per q-group -> partial out^T [D, S] f32,
    summed + transposed on host.

No collectives; host gathers/assembles the full outputs.
"""

import os
import numpy as np
import ml_dtypes

import concourse.bass as bass
import concourse.mybir as mybir
import concourse.tile as tile
from concourse.bass_utils import run_bass_kernel_spmd
from concourse.masks import make_identity

B, S, D, H = 2, 2048, 1024, 16
HD = D // H            # 64
NCORES = 8
HPC = 4                # heads per core
CD = HPC * HD          # 256 ctx dims per core
NT = S // 128          # 16 q-tiles of 128 rows
NC512 = S // 512       # 4 column blocks of 512
NJ = S // 128          # 16 column k-tiles of 128
FP16_MIN = -65504.0

BF16 = mybir.dt.bfloat16
F32 = mybir.dt.float32
NPBF16 = ml_dtypes.bfloat16
AF = mybir.ActivationFunctionType
ALU = mybir.AluOpType
AX = mybir.AxisListType

DEFAULT_OPTS = dict(
    e_bufs=4, pb_bufs=4, pf_bufs=3, pt_bufs=3, sc_bufs=3, cps_bufs=2,
    po_bufs=1, ot_bufs=4, small_bufs=10, probs_eng="sync",
    transp_mode="pe", pst_bufs=2,
)


def _split_excess_waits(nc):
    """walrus in this container rejects >1 sync-wait per instruction
    ("Too many sync wait commands" in CoreV3 setupSyncWait). Move excess
    waits onto NoOps inserted just before the offending instruction."""
    n = 0
    for fn in nc.m.functions:
        for blk in fn.blocks:
            out = []
            for inst in blk.instructions:
                si = inst.sync_info
                if si is not None and si.on_wait and len(si.on_wait) > 1:
                    waits = list(si.on_wait)
                    for w in waits[:-1]:
                        nop = mybir.InstNoOp(name=f"WSPLIT{n}", ins=[], outs=[])
                        n += 1
                        nop.engine = inst.engine
                        nop.sync_info = mybir.SyncInfo(on_wait=[w], on_update=[])
                        out.append(nop)
                    si.on_wait = [waits[-1]]
                out.append(inst)
            blk.instructions[:] = out
    return nc


def _classify(mask01):
    """mask01: [B, S, S] bool. Returns (cls [NT,NC512] in {0,1,2},
    a128 [NT,NJ] bool), merged across batches so one SPMD program
    serves every core."""
    tb = mask01.reshape(B, NT, 128, NC512, 512)
    anyb = tb.any(axis=(2, 4)).any(axis=0)        # [NT, NC512]
    allb = tb.all(axis=(2, 4)).all(axis=0)
    cls = np.where(anyb, np.where(allb, 1, 2), 0).astype(np.int64)
    a128 = mask01.reshape(B, NT, 128, NJ, 128).any(axis=(0, 2, 4))  # [NT, NJ]
    return cls, a128


def _runs(cols):
    """Maximal runs of consecutive ints: [0,1,3] -> [(0,2),(3,4)]."""
    runs = []
    for c in cols:
        if runs and runs[-1][1] == c:
            runs[-1][1] = c + 1
        else:
            runs.append([c, c + 1])
    return [tuple(r) for r in runs]


def build_program(cls, a128, nmix, opts=None):
    o = dict(DEFAULT_OPTS)
    if opts:
        o.update(opts)
    cls = np.asarray(cls)
    a128 = np.asarray(a128)
    active512 = [[c for c in range(NC512) if cls[t, c]] for t in range(NT)]
    mixed_idx = {}
    for t in range(NT):
        for c in range(NC512):
            if cls[t, c] == 2:
                mixed_idx[(t, c)] = len(mixed_idx)
    assert len(mixed_idx) == nmix
    jgroups = [
        sorted({j for t in range(4 * g, 4 * g + 4) for j in range(NJ) if a128[t, j]})
        for g in range(4)
    ]
    a128runs = [_runs([j for j in range(NJ) if a128[t, j]]) for t in range(NT)]

    nc = bass.Bass()
    xqT = nc.dram_tensor("xqT", [D, S], BF16, kind="ExternalInput")
    xkT = nc.dram_tensor("xkT", [D, S], BF16, kind="ExternalInput")
    xvT = nc.dram_tensor("xvT", [D, S], BF16, kind="ExternalInput")
    wq = nc.dram_tensor("wq", [D, CD], BF16, kind="ExternalInput")
    wk = nc.dram_tensor("wk", [D, CD], BF16, kind="ExternalInput")
    wv = nc.dram_tensor("wv", [D, CD], BF16, kind="ExternalInput")
    wo = nc.dram_tensor("wo", [CD, D], BF16, kind="ExternalInput")
    bqs = nc.dram_tensor("bqs", [CD], F32, kind="ExternalInput")   # bq slice * 0.125
    bks = nc.dram_tensor("bks", [CD], F32, kind="ExternalInput")
    bvs = nc.dram_tensor("bvs", [CD], F32, kind="ExternalInput")
    maskb = None
    if nmix:
        maskb = nc.dram_tensor("maskb", [nmix, 128, 512], BF16, kind="ExternalInput")
    probs_o = nc.dram_tensor("probs", [HPC, S, S], F32, kind="ExternalOutput")
    outT_o = nc.dram_tensor("outT", [D, S], F32, kind="ExternalOutput")

    probs_dma_eng = getattr(nc, {"scalar": "scalar", "sync": "sync"}[o["probs_eng"]])

    with tile.TileContext(nc) as tc:
        with (
            tc.tile_pool(name="persist", bufs=1) as pp,
            tc.tile_pool(name="qkv", bufs=1) as qkvp,
        ):
            # ---- persistent loads -------------------------------------
            wq_sb = pp.tile([128, 8, CD], BF16)
            nc.sync.dma_start(out=wq_sb, in_=wq[:, :].rearrange("(a p) n -> p a n", p=128))
            wk_sb = pp.tile([128, 8, CD], BF16)
            nc.sync.dma_start(out=wk_sb, in_=wk[:, :].rearrange("(a p) n -> p a n", p=128))
            wv_sb = pp.tile([128, 8, CD], BF16)
            nc.sync.dma_start(out=wv_sb, in_=wv[:, :].rearrange("(a p) n -> p a n", p=128))
            wo_sb = pp.tile([128, 2, D], BF16)
            nc.sync.dma_start(out=wo_sb, in_=wo[:, :].rearrange("(a p) n -> p a n", p=128))
            bq_sb = pp.tile([128, 2], F32)
            nc.sync.dma_start(out=bq_sb, in_=bqs[:].rearrange("(a p) -> p a", p=128))
            bk_sb = pp.tile([128, 2], F32)
            nc.sync.dma_start(out=bk_sb, in_=bks[:].rearrange("(a p) -> p a", p=128))
            bv_sb = pp.tile([128, CD], F32)
            nc.sync.dma_start(out=bv_sb, in_=bvs[None, :].to_broadcast((128, CD)))
            mask_sb = None
            if nmix:
                mask_sb = pp.tile([128, nmix, 512], BF16)
                nc.sync.dma_start(out=mask_sb, in_=maskb[:, :, :].rearrange("a p n -> p a n"))
            ident = None
            if o["transp_mode"] == "pe":
                ident = pp.tile([128, 128], BF16)
                make_identity(nc, ident)

            # head-pair tiles: pair p holds heads 2p (parts 0-63), 2p+1 (64-127)
            qtp = [qkvp.tile([128, S], BF16, name=f"qtp{p}") for p in range(2)]
            ktp = [qkvp.tile([128, S], BF16, name=f"ktp{p}") for p in range(2)]
            v_sb = qkvp.tile([128, NJ, CD], BF16)
            ctxTp = [qkvp.tile([128, S], BF16, name=f"ctxTp{p}") for p in range(2)]

            # ---- phase A: projections ---------------------------------
            with (
                tc.tile_pool(name="xin", bufs=2) as xp,
                tc.psum_pool(name="psA", bufs=4) as psA,
                tc.psum_pool(name="psV", bufs=2) as psV,
            ):
                for xin, wsb, bsb, outs, scale in (
                    (xqT, wq_sb, bq_sb, qtp, 0.125),
                    (xkT, wk_sb, bk_sb, ktp, 1.0),
                ):
                    x_sb = xp.tile([128, 8, S], BF16, name="x_sb")
                    nc.sync.dma_start(out=x_sb, in_=xin[:, :].rearrange("(a p) n -> p a n", p=128))
                    for ng in range(4):
                        for p in range(2):
                            ps = psA.tile([128, 512], F32, name="psqk")
                            for kt in range(8):
                                nc.tensor.matmul(
                                    ps,
                                    wsb[:, kt, p * 128:(p + 1) * 128],
                                    x_sb[:, kt, ng * 512:(ng + 1) * 512],
                                    start=(kt == 0),
                                    stop=(kt == 7),
                                )
                            nc.scalar.activation(
                                out=outs[p][:, ng * 512:(ng + 1) * 512],
                                in_=ps,
                                func=AF.Identity,
                                bias=bsb[:, p:p + 1],
                                scale=scale,
                            )
                # V projection: natural layout [keys, CD]
                xv_sb = xp.tile([128, 8, S], BF16, name="x_sb")
                nc.sync.dma_start(out=xv_sb, in_=xvT[:, :].rearrange("(a p) n -> p a n", p=128))
                for j in range(NJ):
                    ps = psV.tile([128, CD], F32, name="psv")
                    for kt in range(8):
                        nc.tensor.matmul(
                            ps,
                            xv_sb[:, kt, j * 128:(j + 1) * 128],
                            wv_sb[:, kt, :],
                            start=(kt == 0),
                            stop=(kt == 7),
                        )
                    nc.vector.tensor_add(v_sb[:, j, :], ps, bv_sb)

            # ---- phase B: attention + fused out-projection per q-group -
            with (
                tc.tile_pool(name="work", bufs=3) as wkp,
                tc.tile_pool(name="small", bufs=o["small_bufs"]) as smp,
                tc.tile_pool(name="pt", bufs=o["pt_bufs"]) as ptp,
                tc.tile_pool(name="oute", bufs=o["ot_bufs"]) as op,
                tc.psum_pool(name="psB", bufs=o["sc_bufs"]) as psB,
                tc.psum_pool(name="psT", bufs=o["pst_bufs"]) as psT,
                tc.psum_pool(name="psC", bufs=o["cps_bufs"]) as psC,
                tc.psum_pool(name="psD", bufs=o["po_bufs"]) as psD,
            ):
                for g in range(4):
                    jg = jgroups[g]
                    gs = slice(g * 512, (g + 1) * 512)
                    cps = {}
                    for h in range(HPC):
                        p, hh = h // 2, h % 2
                        hs = slice(hh * 64, (hh + 1) * 64)
                        # probs^T for (h, g): [k-part, j, tl, q128]
                        PT = ptp.tile([128, NJ, 4, 128], BF16, name="PT", tag="PT")
                        for t in range(4 * g, 4 * g + 4):
                            tl = t - 4 * g
                            acts = active512[t]
                            Pb = None
                            if acts:
                                E = wkp.tile([128, S], BF16, name="E", tag="E",
                                             bufs=o["e_bufs"])
                                sums = smp.tile([128, NC512], F32, name="sums", tag="sums")
                                for ci, c in enumerate(acts):
                                    sc = psB.tile([128, 512], F32, name="sc", tag="sc")
                                    nc.tensor.matmul(
                                        sc,
                                        qtp[p][hs, t * 128:(t + 1) * 128],
                                        ktp[p][hs, c * 512:(c + 1) * 512],
                                        start=True,
                                        stop=True,
                                    )
                                    nc.scalar.activation(
                                        out=E[:, c * 512:(c + 1) * 512],
                                        in_=sc,
                                        func=AF.Exp,
                                        accum_out=sums[:, ci:ci + 1],
                                    )
                                for ci, c in enumerate(acts):
                                    if cls[t, c] == 2:
                                        mi = mixed_idx[(t, c)]
                                        cs = slice(c * 512, (c + 1) * 512)
                                        nc.vector.scalar_tensor_tensor(
                                            out=E[:, cs],
                                            in0=E[:, cs],
                                            scalar=0.0,
                                            in1=mask_sb[:, mi, :],
                                            op0=ALU.bypass,
                                            op1=ALU.mult,
                                            accum_out=sums[:, ci:ci + 1],
                                        )
                                stot = smp.tile([128, 1], F32, name="stot", tag="stot")
                                nc.vector.reduce_sum(stot, sums[:, 0:len(acts)], axis=AX.X)
                                rec = smp.tile([128, 1], F32, name="rec", tag="rec")
                                nc.vector.reciprocal(rec, stot)
                                Pb = wkp.tile([128, S], BF16, name="Pb", tag="Pb",
                                              bufs=o["pb_bufs"])
                                Pf = wkp.tile([128, S], F32, name="Pf", tag="Pf",
                                              bufs=o["pf_bufs"])
                                for c0, c1 in _runs(acts):
                                    rs = slice(c0 * 512, c1 * 512)
                                    nc.vector.tensor_scalar_mul(Pb[:, rs], E[:, rs], rec)
                                    nc.vector.tensor_scalar_mul(Pf[:, rs], E[:, rs], rec)
                                    probs_dma_eng.dma_start(
                                        out=probs_o[h, t * 128:(t + 1) * 128, rs],
                                        in_=Pf[:, rs],
                                    )
                            # per-block transpose: PT[:, j, tl, :] =
                            # Pb[:, j*128:(j+1)*128].T for each active j
                            if o["transp_mode"] == "dma":
                                for j0, j1 in (a128runs[t] if acts else []):
                                    nc.sync.dma_start_transpose(
                                        out=PT[:, j0:j1, tl, :],
                                        in_=Pb[:, j0 * 128:j1 * 128],
                                    )
                            else:
                                # PE transposes, 4 blocks batched per PSUM bank
                                for j0, j1 in (a128runs[t] if acts else []):
                                    for c0 in range(j0, j1, 4):
                                        c1 = min(c0 + 4, j1)
                                        pst = psT.tile([128, 4, 128], BF16,
                                                       name="pst", tag="pst")
                                        for jj in range(c0, c1):
                                            nc.tensor.transpose(
                                                pst[:, jj - c0, :],
                                                Pb[:, jj * 128:(jj + 1) * 128],
                                                ident,
                                            )
                                        nc.any.tensor_copy(
                                            PT[:, c0:c1, tl, :],
                                            pst[:, 0:c1 - c0, :],
                                        )
                            inact = [j for j in jg if not (acts and a128[t, j])]
                            for j0, j1 in _runs(inact):
                                nc.vector.memset(PT[:, j0:j1, tl, :], 0.0)
                        # ctx^T: both pair-halves accumulate into cps[p]
                        if hh == 0:
                            cps[p] = psC.tile([128, 512], F32, name="cps", tag="cps")
                        if jg:
                            for idx, j in enumerate(jg):
                                nc.tensor.matmul(
                                    cps[p][hs, :],
                                    v_sb[:, j, h * 64:(h + 1) * 64],
                                    PT[:, j, :, :],
                                    start=(idx == 0),
                                    stop=(idx == len(jg) - 1),
                                )
                        else:
                            nc.vector.memset(cps[p][hs, :], 0.0)
                        if hh == 1:
                            nc.scalar.activation(out=ctxTp[p][:, gs], in_=cps[p], func=AF.Copy)
                    # fused output projection for this q-group
                    for m in range(8):
                        po = psD.tile([128, 512], F32, name="po", tag="po")
                        for p in range(2):
                            nc.tensor.matmul(
                                po,
                                wo_sb[:, p, m * 128:(m + 1) * 128],
                                ctxTp[p][:, gs],
                                start=(p == 0),
                                stop=(p == 1),
                            )
                        ot = op.tile([128, 512], F32, name="ot", tag="ot")
                        nc.scalar.activation(out=ot, in_=po, func=AF.Copy)
                        nc.sync.dma_start(
                            out=outT_o[m * 128:(m + 1) * 128, gs],
                            in_=ot,
                        )

    return _split_excess_waits(nc)


_prog_cache = {}


def kernel(x_q, x_k, x_v, mask, Wq, bq, Wk, bk, Wv, bv, Wo, bo):
    x_q = np.asarray(x_q, np.float32)
    x_k = np.asarray(x_k, np.float32)
    x_v = np.asarray(x_v, np.float32)
    mask = np.asarray(mask)
    Wq = np.asarray(Wq, np.float32)
    Wk = np.asarray(Wk, np.float32)
    Wv = np.asarray(Wv, np.float32)
    Wo = np.asarray(Wo, np.float32)
    bq = np.asarray(bq, np.float32)
    bk = np.asarray(bk, np.float32)
    bv = np.asarray(bv, np.float32)
    bo = np.asarray(bo, np.float32)

    mask01 = (mask != 0).reshape(B, S, S)
    cls, a128 = _classify(mask01)
    mixed = [(t, c) for t in range(NT) for c in range(NC512) if cls[t, c] == 2]
    nmix = len(mixed)

    key = (cls.tobytes(), a128.tobytes())
    if key not in _prog_cache:
        _prog_cache[key] = build_program(cls, a128, nmix)
    nc = _prog_cache[key]

    # host-side sharding / preprocessing
    xT = {}
    for name, x in (("xqT", x_q), ("xkT", x_k), ("xvT", x_v)):
        xT[name] = [np.ascontiguousarray(x[b].T).astype(NPBF16) for b in range(B)]
    if nmix:
        maskb = [
            np.stack(
                [
                    mask01[b, t * 128:(t + 1) * 128, c * 512:(c + 1) * 512]
                    for (t, c) in mixed
                ]
            ).astype(NPBF16)
            for b in range(B)
        ]

    in_maps = []
    for c in range(NCORES):
        b, hg = c // 4, c % 4
        cs = slice(hg * CD, (hg + 1) * CD)
        d = {
            "xqT": xT["xqT"][b],
            "xkT": xT["xkT"][b],
            "xvT": xT["xvT"][b],
            "wq": np.ascontiguousarray(Wq[:, cs]).astype(NPBF16),
            "wk": np.ascontiguousarray(Wk[:, cs]).astype(NPBF16),
            "wv": np.ascontiguousarray(Wv[:, cs]).astype(NPBF16),
            "wo": np.ascontiguousarray(Wo[cs, :]).astype(NPBF16),
            "bqs": np.ascontiguousarray(bq[cs]) * np.float32(0.125),
            "bks": np.ascontiguousarray(bk[cs]),
            "bvs": np.ascontiguousarray(bv[cs]),
        }
        if nmix:
            d["maskb"] = maskb[b]
        in_maps.append(d)

    trace = bool(int(os.environ.get("KERNEL_TRACE", "0")))
    res = run_bass_kernel_spmd(nc, in_maps, core_ids=list(range(NCORES)), trace=trace)
    if trace and res.exec_time_ns is not None:
        print(f"HW exec time: {res.exec_time_ns} ns")
        kernel.last_exec_time_ns = res.exec_time_ns
        kernel.last_trace = res.instructions_and_trace

    probs = np.zeros((B, H, S, S), np.float32)
    out = np.zeros((B, S, D), np.float32)
    for c in range(NCORES):
        b, hg = c // 4, c % 4
        r = res.results[c]
        probs[b, hg * HPC:(hg + 1) * HPC] = r["probs"]
        out[b] += r["outT"].T
    out += bo
    return out, probs


# revision 15
# speedup vs baseline: 1.9778x; 1.0489x over previous
"""Multi-head causal attention on 8 Trainium2 NeuronCores (Bass/Tile).

Problem: B=2, S=2048, D=1024, H=16 heads (HD=64). Reference returns
(out [B,S,D] f32, probs [B,H,S,S] f32).

Sharding (data + head parallel): core c in 0..7 handles batch b=c//4 and
head-group hg=c%4 (4 of 16 heads). Each core:
  - projects Q^T/K^T head-PAIRED ([128, 2048] bf16: pair p holds heads
    2p,2p+1 on partition halves; d-on-partitions) and V ([2048, 256] bf16)
    from host-pre-transposed bf16 activations and column-sliced weights,
  - computes scores = (Q/8).K^T per head (K=64 matmuls reading the pair
    tile's partition half) with block sparsity from the mask (512-wide
    column blocks classified zero/ones/mixed),
  - softmax WITHOUT max-subtraction (scores are O(1) here, exp cannot
    overflow); mask applied as a post-exp multiply so masked probs are
    exactly 0, matching the reference's exp(-65504)->0,
  - writes its [4, S, S] f32 probs slice (fully-masked blocks are skipped;
    output buffers are pre-zeroed by the runtime),
  - ctx via batched DMA-transposed bf16 probs against V (both pair-heads
    accumulate into one PSUM tile), then K=128 output projection with
    row-sliced Wo fused per q-group -> partial out^T [D, S] f32,
    summed + transposed on host.

No collectives; host gathers/assembles the full outputs.
"""

import os
import numpy as np
import ml_dtypes

import concourse.bass as bass
import concourse.mybir as mybir
import concourse.tile as tile
from concourse.bass_utils import run_bass_kernel_spmd
from concourse.masks import make_identity

B, S, D, H = 2, 2048, 1024, 16
HD = D // H            # 64
NCORES = 8
HPC = 4                # heads per core
CD = HPC * HD          # 256 ctx dims per core
NT = S // 128          # 16 q-tiles of 128 rows
NC512 = S // 512       # 4 column blocks of 512
NJ = S // 128          # 16 column k-tiles of 128
FP16_MIN = -65504.0

BF16 = mybir.dt.bfloat16
F32 = mybir.dt.float32
NPBF16 = ml_dtypes.bfloat16
AF = mybir.ActivationFunctionType
ALU = mybir.AluOpType
AX = mybir.AxisListType

DEFAULT_OPTS = dict(
    e_bufs=4, pb_bufs=4, pf_bufs=3, pt_bufs=3, sc_bufs=4, cps_bufs=1,
    po_bufs=1, ot_bufs=4, small_bufs=10, probs_eng="sync",
    transp_mode="pe", pst_bufs=2,
)


def _split_excess_waits(nc):
    """walrus in this container rejects >1 sync-wait per instruction
    ("Too many sync wait commands" in CoreV3 setupSyncWait). Move excess
    waits onto NoOps inserted just before the offending instruction."""
    n = 0
    for fn in nc.m.functions:
        for blk in fn.blocks:
            out = []
            for inst in blk.instructions:
                si = inst.sync_info
                if si is not None and si.on_wait and len(si.on_wait) > 1:
                    waits = list(si.on_wait)
                    for w in waits[:-1]:
                        nop = mybir.InstNoOp(name=f"WSPLIT{n}", ins=[], outs=[])
                        n += 1
                        nop.engine = inst.engine
                        nop.sync_info = mybir.SyncInfo(on_wait=[w], on_update=[])
                        out.append(nop)
                    si.on_wait = [waits[-1]]
                out.append(inst)
            blk.instructions[:] = out
    return nc


def _classify(mask01):
    """mask01: [B, S, S] bool. Returns (cls [NT,NC512] in {0,1,2},
    a128 [NT,NJ] bool), merged across batches so one SPMD program
    serves every core."""
    tb = mask01.reshape(B, NT, 128, NC512, 512)
    anyb = tb.any(axis=(2, 4)).any(axis=0)        # [NT, NC512]
    allb = tb.all(axis=(2, 4)).all(axis=0)
    cls = np.where(anyb, np.where(allb, 1, 2), 0).astype(np.int64)
    a128 = mask01.reshape(B, NT, 128, NJ, 128).any(axis=(0, 2, 4))  # [NT, NJ]
    return cls, a128


def _runs(cols):
    """Maximal runs of consecutive ints: [0,1,3] -> [(0,2),(3,4)]."""
    runs = []
    for c in cols:
        if runs and runs[-1][1] == c:
            runs[-1][1] = c + 1
        else:
            runs.append([c, c + 1])
    return [tuple(r) for r in runs]


def build_program(cls, a128, nmix, opts=None):
    o = dict(DEFAULT_OPTS)
    if opts:
        o.update(opts)
    cls = np.asarray(cls)
    a128 = np.asarray(a128)
    active512 = [[c for c in range(NC512) if cls[t, c]] for t in range(NT)]
    mixed_idx = {}
    for t in range(NT):
        for c in range(NC512):
            if cls[t, c] == 2:
                mixed_idx[(t, c)] = len(mixed_idx)
    assert len(mixed_idx) == nmix
    jgroups = [
        sorted({j for t in range(4 * g, 4 * g + 4) for j in range(NJ) if a128[t, j]})
        for g in range(4)
    ]
    a128runs = [_runs([j for j in range(NJ) if a128[t, j]]) for t in range(NT)]

    nc = bass.Bass()
    xqT = nc.dram_tensor("xqT", [D, S], BF16, kind="ExternalInput")
    xkT = nc.dram_tensor("xkT", [D, S], BF16, kind="ExternalInput")
    xvT = nc.dram_tensor("xvT", [D, S], BF16, kind="ExternalInput")
    wq = nc.dram_tensor("wq", [D, CD], BF16, kind="ExternalInput")
    wk = nc.dram_tensor("wk", [D, CD], BF16, kind="ExternalInput")
    wv = nc.dram_tensor("wv", [D, CD], BF16, kind="ExternalInput")
    wo = nc.dram_tensor("wo", [CD, D], BF16, kind="ExternalInput")
    bqs = nc.dram_tensor("bqs", [CD], F32, kind="ExternalInput")   # bq slice * 0.125
    bks = nc.dram_tensor("bks", [CD], F32, kind="ExternalInput")
    bvs = nc.dram_tensor("bvs", [CD], F32, kind="ExternalInput")
    maskb = None
    if nmix:
        maskb = nc.dram_tensor("maskb", [nmix, 128, 512], BF16, kind="ExternalInput")
    probs_o = nc.dram_tensor("probs", [HPC, S, S], F32, kind="ExternalOutput")
    outT_o = nc.dram_tensor("outT", [D, S], F32, kind="ExternalOutput")

    probs_dma_eng = getattr(nc, {"scalar": "scalar", "sync": "sync"}[o["probs_eng"]])

    with tile.TileContext(nc) as tc:
        with (
            tc.tile_pool(name="persist", bufs=1) as pp,
            tc.tile_pool(name="qkv", bufs=1) as qkvp,
        ):
            # ---- persistent loads -------------------------------------
            wq_sb = pp.tile([128, 8, CD], BF16)
            nc.sync.dma_start(out=wq_sb, in_=wq[:, :].rearrange("(a p) n -> p a n", p=128))
            wk_sb = pp.tile([128, 8, CD], BF16)
            nc.sync.dma_start(out=wk_sb, in_=wk[:, :].rearrange("(a p) n -> p a n", p=128))
            wv_sb = pp.tile([128, 8, CD], BF16)
            nc.sync.dma_start(out=wv_sb, in_=wv[:, :].rearrange("(a p) n -> p a n", p=128))
            wo_sb = pp.tile([128, 2, D], BF16)
            nc.sync.dma_start(out=wo_sb, in_=wo[:, :].rearrange("(a p) n -> p a n", p=128))
            bq_sb = pp.tile([128, 2], F32)
            nc.sync.dma_start(out=bq_sb, in_=bqs[:].rearrange("(a p) -> p a", p=128))
            bk_sb = pp.tile([128, 2], F32)
            nc.sync.dma_start(out=bk_sb, in_=bks[:].rearrange("(a p) -> p a", p=128))
            bv_sb = pp.tile([128, CD], F32)
            nc.sync.dma_start(out=bv_sb, in_=bvs[None, :].to_broadcast((128, CD)))
            mask_sb = None
            if nmix:
                mask_sb = pp.tile([128, nmix, 512], BF16)
                nc.sync.dma_start(out=mask_sb, in_=maskb[:, :, :].rearrange("a p n -> p a n"))
            ident = None
            if o["transp_mode"] == "pe":
                ident = pp.tile([128, 128], BF16)
                make_identity(nc, ident)

            # head-pair tiles: pair p holds heads 2p (parts 0-63), 2p+1 (64-127)
            qtp = [qkvp.tile([128, S], BF16, name=f"qtp{p}") for p in range(2)]
            ktp = [qkvp.tile([128, S], BF16, name=f"ktp{p}") for p in range(2)]
            v_sb = qkvp.tile([128, NJ, CD], BF16)
            ctxTp = [qkvp.tile([128, S], BF16, name=f"ctxTp{p}") for p in range(2)]

            # ---- phase A: projections ---------------------------------
            with (
                tc.tile_pool(name="xin", bufs=2) as xp,
                tc.psum_pool(name="psA", bufs=4) as psA,
                tc.psum_pool(name="psV", bufs=2) as psV,
            ):
                for xin, wsb, bsb, outs, scale in (
                    (xqT, wq_sb, bq_sb, qtp, 0.125),
                    (xkT, wk_sb, bk_sb, ktp, 1.0),
                ):
                    x_sb = xp.tile([128, 8, S], BF16, name="x_sb")
                    xr = xin[:, :].rearrange("(a p) n -> p a n", p=128)
                    for kt in range(8):
                        nc.sync.dma_start(out=x_sb[:, kt, :], in_=xr[:, kt, :])
                    for ng in range(4):
                        for p in range(2):
                            ps = psA.tile([128, 512], F32, name="psqk")
                            for kt in range(8):
                                nc.tensor.matmul(
                                    ps,
                                    wsb[:, kt, p * 128:(p + 1) * 128],
                                    x_sb[:, kt, ng * 512:(ng + 1) * 512],
                                    start=(kt == 0),
                                    stop=(kt == 7),
                                )
                            nc.scalar.activation(
                                out=outs[p][:, ng * 512:(ng + 1) * 512],
                                in_=ps,
                                func=AF.Identity,
                                bias=bsb[:, p:p + 1],
                                scale=scale,
                            )
                # V projection: natural layout [keys, CD]
                xv_sb = xp.tile([128, 8, S], BF16, name="x_sb")
                xvr = xvT[:, :].rearrange("(a p) n -> p a n", p=128)
                for kt in range(8):
                    nc.sync.dma_start(out=xv_sb[:, kt, :], in_=xvr[:, kt, :])
                for j in range(NJ):
                    ps = psV.tile([128, CD], F32, name="psv")
                    for kt in range(8):
                        nc.tensor.matmul(
                            ps,
                            xv_sb[:, kt, j * 128:(j + 1) * 128],
                            wv_sb[:, kt, :],
                            start=(kt == 0),
                            stop=(kt == 7),
                        )
                    nc.vector.tensor_add(v_sb[:, j, :], ps, bv_sb)

            # ---- phase B: attention + fused out-projection per q-group -
            with (
                tc.tile_pool(name="work", bufs=3) as wkp,
                tc.tile_pool(name="small", bufs=o["small_bufs"]) as smp,
                tc.tile_pool(name="pt", bufs=o["pt_bufs"]) as ptp,
                tc.tile_pool(name="oute", bufs=o["ot_bufs"]) as op,
                tc.psum_pool(name="psB", bufs=o["sc_bufs"]) as psB,
                tc.psum_pool(name="psT", bufs=o["pst_bufs"]) as psT,
                tc.psum_pool(name="psC", bufs=o["cps_bufs"]) as psC,
                tc.psum_pool(name="psD", bufs=o["po_bufs"]) as psD,
            ):
                for g in range(4):
                    jg = jgroups[g]
                    gs = slice(g * 512, (g + 1) * 512)
                    cps = {}
                    for h in range(HPC):
                        p, hh = h // 2, h % 2
                        hs = slice(hh * 64, (hh + 1) * 64)
                        # probs^T for (h, g): [k-part, j, tl, q128]
                        PT = ptp.tile([128, NJ, 4, 128], BF16, name="PT", tag="PT")
                        for t in range(4 * g, 4 * g + 4):
                            tl = t - 4 * g
                            acts = active512[t]
                            Pb = None
                            if acts:
                                E = wkp.tile([128, S], BF16, name="E", tag="E",
                                             bufs=o["e_bufs"])
                                sums = smp.tile([128, NC512], F32, name="sums", tag="sums")
                                for ci, c in enumerate(acts):
                                    sc = psB.tile([128, 512], F32, name="sc", tag="sc")
                                    nc.tensor.matmul(
                                        sc,
                                        qtp[p][hs, t * 128:(t + 1) * 128],
                                        ktp[p][hs, c * 512:(c + 1) * 512],
                                        start=True,
                                        stop=True,
                                    )
                                    nc.scalar.activation(
                                        out=E[:, c * 512:(c + 1) * 512],
                                        in_=sc,
                                        func=AF.Exp,
                                        accum_out=sums[:, ci:ci + 1],
                                    )
                                for ci, c in enumerate(acts):
                                    if cls[t, c] == 2:
                                        mi = mixed_idx[(t, c)]
                                        cs = slice(c * 512, (c + 1) * 512)
                                        nc.vector.scalar_tensor_tensor(
                                            out=E[:, cs],
                                            in0=E[:, cs],
                                            scalar=0.0,
                                            in1=mask_sb[:, mi, :],
                                            op0=ALU.bypass,
                                            op1=ALU.mult,
                                            accum_out=sums[:, ci:ci + 1],
                                        )
                                stot = smp.tile([128, 1], F32, name="stot", tag="stot")
                                nc.vector.reduce_sum(stot, sums[:, 0:len(acts)], axis=AX.X)
                                rec = smp.tile([128, 1], F32, name="rec", tag="rec")
                                nc.vector.reciprocal(rec, stot)
                                Pb = wkp.tile([128, S], BF16, name="Pb", tag="Pb",
                                              bufs=o["pb_bufs"])
                                Pf = wkp.tile([128, S], F32, name="Pf", tag="Pf",
                                              bufs=o["pf_bufs"])
                                for c0, c1 in _runs(acts):
                                    rs = slice(c0 * 512, c1 * 512)
                                    nc.vector.tensor_scalar_mul(Pb[:, rs], E[:, rs], rec)
                                    nc.vector.tensor_scalar_mul(Pf[:, rs], E[:, rs], rec)
                                    probs_dma_eng.dma_start(
                                        out=probs_o[h, t * 128:(t + 1) * 128, rs],
                                        in_=Pf[:, rs],
                                    )
                            # per-block transpose: PT[:, j, tl, :] =
                            # Pb[:, j*128:(j+1)*128].T for each active j
                            if o["transp_mode"] == "dma":
                                for j0, j1 in (a128runs[t] if acts else []):
                                    nc.sync.dma_start_transpose(
                                        out=PT[:, j0:j1, tl, :],
                                        in_=Pb[:, j0 * 128:j1 * 128],
                                    )
                            else:
                                # PE transposes, 4 blocks batched per PSUM bank
                                for j0, j1 in (a128runs[t] if acts else []):
                                    for c0 in range(j0, j1, 4):
                                        c1 = min(c0 + 4, j1)
                                        pst = psT.tile([128, 4, 128], BF16,
                                                       name="pst", tag="pst")
                                        for jj in range(c0, c1):
                                            nc.tensor.transpose(
                                                pst[:, jj - c0, :],
                                                Pb[:, jj * 128:(jj + 1) * 128],
                                                ident,
                                            )
                                        nc.any.tensor_copy(
                                            PT[:, c0:c1, tl, :],
                                            pst[:, 0:c1 - c0, :],
                                        )
                            inact = [j for j in jg if not (acts and a128[t, j])]
                            for j0, j1 in _runs(inact):
                                nc.vector.memset(PT[:, j0:j1, tl, :], 0.0)
                        # ctx^T: both pair-halves accumulate into cps[p]
                        if hh == 0:
                            cps[p] = psC.tile([128, 512], F32, name="cps", tag="cps")
                        if jg:
                            for idx, j in enumerate(jg):
                                nc.tensor.matmul(
                                    cps[p][hs, :],
                                    v_sb[:, j, h * 64:(h + 1) * 64],
                                    PT[:, j, :, :],
                                    start=(idx == 0),
                                    stop=(idx == len(jg) - 1),
                                )
                        else:
                            nc.vector.memset(cps[p][hs, :], 0.0)
                        if hh == 1:
                            nc.scalar.activation(out=ctxTp[p][:, gs], in_=cps[p], func=AF.Copy)
                    # fused output projection for this q-group
                    for m in range(8):
                        po = psD.tile([128, 512], F32, name="po", tag="po")
                        for p in range(2):
                            nc.tensor.matmul(
                                po,
                                wo_sb[:, p, m * 128:(m + 1) * 128],
                                ctxTp[p][:, gs],
                                start=(p == 0),
                                stop=(p == 1),
                            )
                        ot = op.tile([128, 512], F32, name="ot", tag="ot")
                        nc.scalar.activation(out=ot, in_=po, func=AF.Copy)
                        nc.sync.dma_start(
                            out=outT_o[m * 128:(m + 1) * 128, gs],
                            in_=ot,
                        )

    return _split_excess_waits(nc)


_prog_cache = {}


def kernel(x_q, x_k, x_v, mask, Wq, bq, Wk, bk, Wv, bv, Wo, bo):
    x_q = np.asarray(x_q, np.float32)
    x_k = np.asarray(x_k, np.float32)
    x_v = np.asarray(x_v, np.float32)
    mask = np.asarray(mask)
    Wq = np.asarray(Wq, np.float32)
    Wk = np.asarray(Wk, np.float32)
    Wv = np.asarray(Wv, np.float32)
    Wo = np.asarray(Wo, np.float32)
    bq = np.asarray(bq, np.float32)
    bk = np.asarray(bk, np.float32)
    bv = np.asarray(bv, np.float32)
    bo = np.asarray(bo, np.float32)

    mask01 = (mask != 0).reshape(B, S, S)
    cls, a128 = _classify(mask01)
    mixed = [(t, c) for t in range(NT) for c in range(NC512) if cls[t, c] == 2]
    nmix = len(mixed)

    key = (cls.tobytes(), a128.tobytes())
    if key not in _prog_cache:
        _prog_cache[key] = build_program(cls, a128, nmix)
    nc = _prog_cache[key]

    # host-side sharding / preprocessing
    xT = {}
    for name, x in (("xqT", x_q), ("xkT", x_k), ("xvT", x_v)):
        xT[name] = [np.ascontiguousarray(x[b].T).astype(NPBF16) for b in range(B)]
    if nmix:
        maskb = [
            np.stack(
                [
                    mask01[b, t * 128:(t + 1) * 128, c * 512:(c + 1) * 512]
                    for (t, c) in mixed
                ]
            ).astype(NPBF16)
            for b in range(B)
        ]

    in_maps = []
    for c in range(NCORES):
        b, hg = c // 4, c % 4
        cs = slice(hg * CD, (hg + 1) * CD)
        d = {
            "xqT": xT["xqT"][b],
            "xkT": xT["xkT"][b],
            "xvT": xT["xvT"][b],
            "wq": np.ascontiguousarray(Wq[:, cs]).astype(NPBF16),
            "wk": np.ascontiguousarray(Wk[:, cs]).astype(NPBF16),
            "wv": np.ascontiguousarray(Wv[:, cs]).astype(NPBF16),
            "wo": np.ascontiguousarray(Wo[cs, :]).astype(NPBF16),
            "bqs": np.ascontiguousarray(bq[cs]) * np.float32(0.125),
            "bks": np.ascontiguousarray(bk[cs]),
            "bvs": np.ascontiguousarray(bv[cs]),
        }
        if nmix:
            d["maskb"] = maskb[b]
        in_maps.append(d)

    trace = bool(int(os.environ.get("KERNEL_TRACE", "0")))
    res = run_bass_kernel_spmd(nc, in_maps, core_ids=list(range(NCORES)), trace=trace)
    if trace and res.exec_time_ns is not None:
        print(f"HW exec time: {res.exec_time_ns} ns")
        kernel.last_exec_time_ns = res.exec_time_ns
        kernel.last_trace = res.instructions_and_trace

    probs = np.zeros((B, H, S, S), np.float32)
    out = np.zeros((B, S, D), np.float32)
    for c in range(NCORES):
        b, hg = c // 4, c % 4
        r = res.results[c]
        probs[b, hg * HPC:(hg + 1) * HPC] = r["probs"]
        out[b] += r["outT"].T
    out += bo
    return out, probs


# revision 18
# speedup vs baseline: 2.0561x; 1.0396x over previous
"""Multi-head causal attention on 8 Trainium2 NeuronCores (Bass/Tile).

Problem: B=2, S=2048, D=1024, H=16 heads (HD=64). Reference returns
(out [B,S,D] f32, probs [B,H,S,S] f32).

Sharding (data + head parallel): core c in 0..7 handles batch b=c//4 and
head-group hg=c%4 (4 of 16 heads). Each core:
  - projects Q^T/K^T head-PAIRED ([128, 2048] bf16: pair p holds heads
    2p,2p+1 on partition halves; d-on-partitions) and V ([2048, 256] bf16)
    from host-pre-transposed bf16 activations and column-sliced weights,
  - computes scores = (Q/8).K^T per head (K=64 matmuls reading the pair
    tile's partition half) with block sparsity from the mask (512-wide
    column blocks classified zero/ones/mixed),
  - softmax WITHOUT max-subtraction (scores are O(1) here, exp cannot
    overflow); mask applied as a post-exp multiply so masked probs are
    exactly 0, matching the reference's exp(-65504)->0,
  - writes its [4, S, S] f32 probs slice (fully-masked blocks are skipped;
    output buffers are pre-zeroed by the runtime),
  - ctx via batched DMA-transposed bf16 probs against V (both pair-heads
    accumulate into one PSUM tile), then K=128 output projection with
    row-sliced Wo fused per q-group -> partial out^T [D, S] f32,
    summed + transposed on host.

No collectives; host gathers/assembles the full outputs.
"""

import os
import numpy as np
import ml_dtypes

import concourse.bass as bass
import concourse.mybir as mybir
import concourse.tile as tile
from concourse.bass_utils import run_bass_kernel_spmd
from concourse.masks import make_identity

B, S, D, H = 2, 2048, 1024, 16
HD = D // H            # 64
NCORES = 8
HPC = 4                # heads per core
CD = HPC * HD          # 256 ctx dims per core
NT = S // 128          # 16 q-tiles of 128 rows
NC512 = S // 512       # 4 column blocks of 512
NJ = S // 128          # 16 column k-tiles of 128
FP16_MIN = -65504.0

BF16 = mybir.dt.bfloat16
F32 = mybir.dt.float32
NPBF16 = ml_dtypes.bfloat16
AF = mybir.ActivationFunctionType
ALU = mybir.AluOpType
AX = mybir.AxisListType

DEFAULT_OPTS = dict(
    e_bufs=4, pb_bufs=4, pf_bufs=3, pt_bufs=3, sc_bufs=4, cps_bufs=1,
    po_bufs=1, ot_bufs=4, small_bufs=10, probs_eng="sync",
    transp_mode="pe", pst_bufs=2,
)


def _split_excess_waits(nc):
    """walrus in this container rejects >1 sync-wait per instruction
    ("Too many sync wait commands" in CoreV3 setupSyncWait). Move excess
    waits onto NoOps inserted just before the offending instruction."""
    n = 0
    for fn in nc.m.functions:
        for blk in fn.blocks:
            out = []
            for inst in blk.instructions:
                si = inst.sync_info
                if si is not None and si.on_wait and len(si.on_wait) > 1:
                    waits = list(si.on_wait)
                    for w in waits[:-1]:
                        nop = mybir.InstNoOp(name=f"WSPLIT{n}", ins=[], outs=[])
                        n += 1
                        nop.engine = inst.engine
                        nop.sync_info = mybir.SyncInfo(on_wait=[w], on_update=[])
                        out.append(nop)
                    si.on_wait = [waits[-1]]
                out.append(inst)
            blk.instructions[:] = out
    return nc


def _classify(mask01):
    """mask01: [B, S, S] bool. Returns (cls [NT,NC512] in {0,1,2},
    a128 [NT,NJ] bool), merged across batches so one SPMD program
    serves every core."""
    tb = mask01.reshape(B, NT, 128, NC512, 512)
    anyb = tb.any(axis=(2, 4)).any(axis=0)        # [NT, NC512]
    allb = tb.all(axis=(2, 4)).all(axis=0)
    cls = np.where(anyb, np.where(allb, 1, 2), 0).astype(np.int64)
    a128 = mask01.reshape(B, NT, 128, NJ, 128).any(axis=(0, 2, 4))  # [NT, NJ]
    return cls, a128


def _runs(cols):
    """Maximal runs of consecutive ints: [0,1,3] -> [(0,2),(3,4)]."""
    runs = []
    for c in cols:
        if runs and runs[-1][1] == c:
            runs[-1][1] = c + 1
        else:
            runs.append([c, c + 1])
    return [tuple(r) for r in runs]


def build_program(cls, a128, nmix, opts=None):
    o = dict(DEFAULT_OPTS)
    if opts:
        o.update(opts)
    cls = np.asarray(cls)
    a128 = np.asarray(a128)
    active512 = [[c for c in range(NC512) if cls[t, c]] for t in range(NT)]
    mixed_idx = {}
    for t in range(NT):
        for c in range(NC512):
            if cls[t, c] == 2:
                mixed_idx[(t, c)] = len(mixed_idx)
    assert len(mixed_idx) == nmix
    jgroups = [
        sorted({j for t in range(4 * g, 4 * g + 4) for j in range(NJ) if a128[t, j]})
        for g in range(4)
    ]
    a128runs = [_runs([j for j in range(NJ) if a128[t, j]]) for t in range(NT)]
    # per-(t, c) active extent in elements within the 512-block: full 512 for
    # interior blocks of a run; the last block of each run is trimmed to the
    # 128-aligned causal frontier (max active j within it).
    bext = np.zeros((NT, NC512), np.int64)
    for t in range(NT):
        for c in range(NC512):
            js = [j for j in range(4 * c, 4 * c + 4) if a128[t, j]]
            if js:
                bext[t, c] = (max(js) + 1) * 128 - c * 512

    nc = bass.Bass()
    xqT = nc.dram_tensor("xqT", [D, S], BF16, kind="ExternalInput")
    xkT = nc.dram_tensor("xkT", [D, S], BF16, kind="ExternalInput")
    xvT = nc.dram_tensor("xvT", [D, S], BF16, kind="ExternalInput")
    wq = nc.dram_tensor("wq", [D, CD], BF16, kind="ExternalInput")
    wk = nc.dram_tensor("wk", [D, CD], BF16, kind="ExternalInput")
    wv = nc.dram_tensor("wv", [D, CD], BF16, kind="ExternalInput")
    wo = nc.dram_tensor("wo", [CD, D], BF16, kind="ExternalInput")
    bqs = nc.dram_tensor("bqs", [CD], F32, kind="ExternalInput")   # bq slice * 0.125
    bks = nc.dram_tensor("bks", [CD], F32, kind="ExternalInput")
    bvs = nc.dram_tensor("bvs", [CD], F32, kind="ExternalInput")
    maskb = None
    if nmix:
        maskb = nc.dram_tensor("maskb", [nmix, 128, 512], BF16, kind="ExternalInput")
    probs_o = nc.dram_tensor("probs", [HPC, S, S], F32, kind="ExternalOutput")
    outT_o = nc.dram_tensor("outT", [D, S], F32, kind="ExternalOutput")

    probs_dma_eng = getattr(nc, {"scalar": "scalar", "sync": "sync"}[o["probs_eng"]])

    with tile.TileContext(nc) as tc:
        with (
            tc.tile_pool(name="persist", bufs=1) as pp,
            tc.tile_pool(name="qkv", bufs=1) as qkvp,
        ):
            # ---- persistent loads -------------------------------------
            wq_sb = pp.tile([128, 8, CD], BF16)
            nc.sync.dma_start(out=wq_sb, in_=wq[:, :].rearrange("(a p) n -> p a n", p=128))
            wk_sb = pp.tile([128, 8, CD], BF16)
            nc.sync.dma_start(out=wk_sb, in_=wk[:, :].rearrange("(a p) n -> p a n", p=128))
            wv_sb = pp.tile([128, 8, CD], BF16)
            nc.sync.dma_start(out=wv_sb, in_=wv[:, :].rearrange("(a p) n -> p a n", p=128))
            wo_sb = pp.tile([128, 2, D], BF16)
            nc.sync.dma_start(out=wo_sb, in_=wo[:, :].rearrange("(a p) n -> p a n", p=128))
            bq_sb = pp.tile([128, 2], F32)
            nc.sync.dma_start(out=bq_sb, in_=bqs[:].rearrange("(a p) -> p a", p=128))
            bk_sb = pp.tile([128, 2], F32)
            nc.sync.dma_start(out=bk_sb, in_=bks[:].rearrange("(a p) -> p a", p=128))
            bv_sb = pp.tile([128, CD], F32)
            nc.sync.dma_start(out=bv_sb, in_=bvs[None, :].to_broadcast((128, CD)))
            mask_sb = None
            if nmix:
                mask_sb = pp.tile([128, nmix, 512], BF16)
                nc.sync.dma_start(out=mask_sb, in_=maskb[:, :, :].rearrange("a p n -> p a n"))
            ident = None
            if o["transp_mode"] == "pe":
                ident = pp.tile([128, 128], BF16)
                make_identity(nc, ident)

            # head-pair tiles: pair p holds heads 2p (parts 0-63), 2p+1 (64-127)
            qtp = [qkvp.tile([128, S], BF16, name=f"qtp{p}") for p in range(2)]
            ktp = [qkvp.tile([128, S], BF16, name=f"ktp{p}") for p in range(2)]
            v_sb = qkvp.tile([128, NJ, CD], BF16)
            ctxTp = [qkvp.tile([128, S], BF16, name=f"ctxTp{p}") for p in range(2)]

            # ---- phase A: projections ---------------------------------
            with (
                tc.tile_pool(name="xin", bufs=2) as xp,
                tc.psum_pool(name="psA", bufs=4) as psA,
                tc.psum_pool(name="psV", bufs=2) as psV,
            ):
                for xin, wsb, bsb, outs, scale in (
                    (xqT, wq_sb, bq_sb, qtp, 0.125),
                    (xkT, wk_sb, bk_sb, ktp, 1.0),
                ):
                    x_sb = xp.tile([128, 8, S], BF16, name="x_sb")
                    xr = xin[:, :].rearrange("(a p) n -> p a n", p=128)
                    for kt in range(8):
                        nc.sync.dma_start(out=x_sb[:, kt, :], in_=xr[:, kt, :])
                    for ng in range(4):
                        for p in range(2):
                            ps = psA.tile([128, 512], F32, name="psqk")
                            for kt in range(8):
                                nc.tensor.matmul(
                                    ps,
                                    wsb[:, kt, p * 128:(p + 1) * 128],
                                    x_sb[:, kt, ng * 512:(ng + 1) * 512],
                                    start=(kt == 0),
                                    stop=(kt == 7),
                                )
                            nc.scalar.activation(
                                out=outs[p][:, ng * 512:(ng + 1) * 512],
                                in_=ps,
                                func=AF.Identity,
                                bias=bsb[:, p:p + 1],
                                scale=scale,
                            )
                # V projection: natural layout [keys, CD]
                xv_sb = xp.tile([128, 8, S], BF16, name="x_sb")
                xvr = xvT[:, :].rearrange("(a p) n -> p a n", p=128)
                for kt in range(8):
                    nc.sync.dma_start(out=xv_sb[:, kt, :], in_=xvr[:, kt, :])
                for j in range(NJ):
                    ps = psV.tile([128, CD], F32, name="psv")
                    for kt in range(8):
                        nc.tensor.matmul(
                            ps,
                            xv_sb[:, kt, j * 128:(j + 1) * 128],
                            wv_sb[:, kt, :],
                            start=(kt == 0),
                            stop=(kt == 7),
                        )
                    nc.vector.tensor_add(v_sb[:, j, :], ps, bv_sb)

            # ---- phase B: attention + fused out-projection per q-group -
            with (
                tc.tile_pool(name="work", bufs=3) as wkp,
                tc.tile_pool(name="small", bufs=o["small_bufs"]) as smp,
                tc.tile_pool(name="pt", bufs=o["pt_bufs"]) as ptp,
                tc.tile_pool(name="oute", bufs=o["ot_bufs"]) as op,
                tc.psum_pool(name="psB", bufs=o["sc_bufs"]) as psB,
                tc.psum_pool(name="psT", bufs=o["pst_bufs"]) as psT,
                tc.psum_pool(name="psC", bufs=o["cps_bufs"]) as psC,
                tc.psum_pool(name="psD", bufs=o["po_bufs"]) as psD,
            ):
                for g in range(4):
                    jg = jgroups[g]
                    gs = slice(g * 512, (g + 1) * 512)
                    cps = {}
                    for h in range(HPC):
                        p, hh = h // 2, h % 2
                        hs = slice(hh * 64, (hh + 1) * 64)
                        # probs^T for (h, g): [k-part, j, tl, q128]
                        PT = ptp.tile([128, NJ, 4, 128], BF16, name="PT", tag="PT")
                        for t in range(4 * g, 4 * g + 4):
                            tl = t - 4 * g
                            acts = active512[t]
                            Pb = None
                            if acts:
                                E = wkp.tile([128, S], BF16, name="E", tag="E",
                                             bufs=o["e_bufs"])
                                sums = smp.tile([128, NC512], F32, name="sums", tag="sums")
                                rr = _runs(acts)
                                rend = {c1 - 1: None for c0, c1 in rr}
                                for ci, c in enumerate(acts):
                                    bx = int(bext[t, c]) if c in rend else 512
                                    sc = psB.tile([128, 512], F32, name="sc", tag="sc")
                                    nc.tensor.matmul(
                                        sc[:, 0:bx],
                                        qtp[p][hs, t * 128:(t + 1) * 128],
                                        ktp[p][hs, c * 512:c * 512 + bx],
                                        start=True,
                                        stop=True,
                                    )
                                    nc.scalar.activation(
                                        out=E[:, c * 512:c * 512 + bx],
                                        in_=sc[:, 0:bx],
                                        func=AF.Exp,
                                        accum_out=sums[:, ci:ci + 1],
                                    )
                                for ci, c in enumerate(acts):
                                    if cls[t, c] == 2:
                                        bx = int(bext[t, c]) if c in rend else 512
                                        mi = mixed_idx[(t, c)]
                                        cs = slice(c * 512, c * 512 + bx)
                                        nc.vector.scalar_tensor_tensor(
                                            out=E[:, cs],
                                            in0=E[:, cs],
                                            scalar=0.0,
                                            in1=mask_sb[:, mi, 0:bx],
                                            op0=ALU.bypass,
                                            op1=ALU.mult,
                                            accum_out=sums[:, ci:ci + 1],
                                        )
                                stot = smp.tile([128, 1], F32, name="stot", tag="stot")
                                nc.vector.reduce_sum(stot, sums[:, 0:len(acts)], axis=AX.X)
                                rec = smp.tile([128, 1], F32, name="rec", tag="rec")
                                nc.vector.reciprocal(rec, stot)
                                Pb = wkp.tile([128, S], BF16, name="Pb", tag="Pb",
                                              bufs=o["pb_bufs"])
                                Pf = wkp.tile([128, S], F32, name="Pf", tag="Pf",
                                              bufs=o["pf_bufs"])
                                for c0, c1 in rr:
                                    rs = slice(c0 * 512, (c1 - 1) * 512 + int(bext[t, c1 - 1]))
                                    nc.vector.tensor_scalar_mul(Pb[:, rs], E[:, rs], rec)
                                    nc.vector.tensor_scalar_mul(Pf[:, rs], E[:, rs], rec)
                                    probs_dma_eng.dma_start(
                                        out=probs_o[h, t * 128:(t + 1) * 128, rs],
                                        in_=Pf[:, rs],
                                    )
                            # per-block transpose: PT[:, j, tl, :] =
                            # Pb[:, j*128:(j+1)*128].T for each active j
                            if o["transp_mode"] == "dma":
                                for j0, j1 in (a128runs[t] if acts else []):
                                    nc.sync.dma_start_transpose(
                                        out=PT[:, j0:j1, tl, :],
                                        in_=Pb[:, j0 * 128:j1 * 128],
                                    )
                            else:
                                # PE transposes, 4 blocks batched per PSUM bank
                                for j0, j1 in (a128runs[t] if acts else []):
                                    for c0 in range(j0, j1, 4):
                                        c1 = min(c0 + 4, j1)
                                        pst = psT.tile([128, 4, 128], BF16,
                                                       name="pst", tag="pst")
                                        for jj in range(c0, c1):
                                            nc.tensor.transpose(
                                                pst[:, jj - c0, :],
                                                Pb[:, jj * 128:(jj + 1) * 128],
                                                ident,
                                            )
                                        nc.any.tensor_copy(
                                            PT[:, c0:c1, tl, :],
                                            pst[:, 0:c1 - c0, :],
                                        )
                            inact = [j for j in jg if not (acts and a128[t, j])]
                            for j0, j1 in _runs(inact):
                                nc.vector.memset(PT[:, j0:j1, tl, :], 0.0)
                        # ctx^T: both pair-halves accumulate into cps[p]
                        if hh == 0:
                            cps[p] = psC.tile([128, 512], F32, name="cps", tag="cps")
                        if jg:
                            for idx, j in enumerate(jg):
                                nc.tensor.matmul(
                                    cps[p][hs, :],
                                    v_sb[:, j, h * 64:(h + 1) * 64],
                                    PT[:, j, :, :],
                                    start=(idx == 0),
                                    stop=(idx == len(jg) - 1),
                                )
                        else:
                            nc.vector.memset(cps[p][hs, :], 0.0)
                        if hh == 1:
                            nc.scalar.activation(out=ctxTp[p][:, gs], in_=cps[p], func=AF.Copy)
                    # fused output projection for this q-group
                    for m in range(8):
                        po = psD.tile([128, 512], F32, name="po", tag="po")
                        for p in range(2):
                            nc.tensor.matmul(
                                po,
                                wo_sb[:, p, m * 128:(m + 1) * 128],
                                ctxTp[p][:, gs],
                                start=(p == 0),
                                stop=(p == 1),
                            )
                        ot = op.tile([128, 512], F32, name="ot", tag="ot")
                        nc.scalar.activation(out=ot, in_=po, func=AF.Copy)
                        nc.sync.dma_start(
                            out=outT_o[m * 128:(m + 1) * 128, gs],
                            in_=ot,
                        )

    return _split_excess_waits(nc)


_prog_cache = {}


def kernel(x_q, x_k, x_v, mask, Wq, bq, Wk, bk, Wv, bv, Wo, bo):
    x_q = np.asarray(x_q, np.float32)
    x_k = np.asarray(x_k, np.float32)
    x_v = np.asarray(x_v, np.float32)
    mask = np.asarray(mask)
    Wq = np.asarray(Wq, np.float32)
    Wk = np.asarray(Wk, np.float32)
    Wv = np.asarray(Wv, np.float32)
    Wo = np.asarray(Wo, np.float32)
    bq = np.asarray(bq, np.float32)
    bk = np.asarray(bk, np.float32)
    bv = np.asarray(bv, np.float32)
    bo = np.asarray(bo, np.float32)

    mask01 = (mask != 0).reshape(B, S, S)
    cls, a128 = _classify(mask01)
    mixed = [(t, c) for t in range(NT) for c in range(NC512) if cls[t, c] == 2]
    nmix = len(mixed)

    key = (cls.tobytes(), a128.tobytes())
    if key not in _prog_cache:
        _prog_cache[key] = build_program(cls, a128, nmix)
    nc = _prog_cache[key]

    # host-side sharding / preprocessing
    xT = {}
    for name, x in (("xqT", x_q), ("xkT", x_k), ("xvT", x_v)):
        xT[name] = [np.ascontiguousarray(x[b].T).astype(NPBF16) for b in range(B)]
    if nmix:
        maskb = [
            np.stack(
                [
                    mask01[b, t * 128:(t + 1) * 128, c * 512:(c + 1) * 512]
                    for (t, c) in mixed
                ]
            ).astype(NPBF16)
            for b in range(B)
        ]

    in_maps = []
    for c in range(NCORES):
        b, hg = c // 4, c % 4
        cs = slice(hg * CD, (hg + 1) * CD)
        d = {
            "xqT": xT["xqT"][b],
            "xkT": xT["xkT"][b],
            "xvT": xT["xvT"][b],
            "wq": np.ascontiguousarray(Wq[:, cs]).astype(NPBF16),
            "wk": np.ascontiguousarray(Wk[:, cs]).astype(NPBF16),
            "wv": np.ascontiguousarray(Wv[:, cs]).astype(NPBF16),
            "wo": np.ascontiguousarray(Wo[cs, :]).astype(NPBF16),
            "bqs": np.ascontiguousarray(bq[cs]) * np.float32(0.125),
            "bks": np.ascontiguousarray(bk[cs]),
            "bvs": np.ascontiguousarray(bv[cs]),
        }
        if nmix:
            d["maskb"] = maskb[b]
        in_maps.append(d)

    trace = bool(int(os.environ.get("KERNEL_TRACE", "0")))
    res = run_bass_kernel_spmd(nc, in_maps, core_ids=list(range(NCORES)), trace=trace)
    if trace and res.exec_time_ns is not None:
        print(f"HW exec time: {res.exec_time_ns} ns")
        kernel.last_exec_time_ns = res.exec_time_ns
        kernel.last_trace = res.instructions_and_trace

    probs = np.zeros((B, H, S, S), np.float32)
    out = np.zeros((B, S, D), np.float32)
    for c in range(NCORES):
        b, hg = c // 4, c % 4
        r = res.results[c]
        probs[b, hg * HPC:(hg + 1) * HPC] = r["probs"]
        out[b] += r["outT"].T
    out += bo
    return out, probs
